# revision 15
# baseline (speedup 1.0000x reference)
"""AutoFormer forward pass on 8 TRN2 NeuronCores (Bass/Tile).

Strategy: data-parallel over batch (2 batches per core). Activations kept
transposed ([D, tokens]) in SBUF. Autocorrelation realized without FFT:

  mean_value[b,l] = (1/512) * sum_t <q[b,(t+l)%L,:], k[b,t,:]>
                  = diagonal sums of the Gram matrix G = Q K^T,
computed by shear-reading a row-doubled copy of G from DRAM (a DMA whose
per-partition stride is rowlen+1 turns circular diagonals into columns),
then a ones-matmul column reduction.

Top-k (k=17) over the batch-mean correlation requires a cross-core
AllReduce of the [1,384] per-core sums (the only collective). The top-17
threshold tau is found with 3 rounds of vector.max + match_replace; the
softmax over selected delays is a masked softmax (select + exp), giving a
sparse weight vector w. The delay-aggregation  out[l] = sum_k w_k
v[(l+d_k)%L]  is a circulant matmul: the circulant S^T[t,l] = w[(l-t)%L]
is materialized by the same DMA shear trick from a DRAM buffer holding w
replicated every 384 elements (stride-383 reads).

Series decomposition (moving average k=25, edge-replicate) is done with a
cumsum scan + window differences + edge-correction STTs.

All matmuls fp32 (exact; fp32r fast path can be enabled later).
"""

import numpy as np
import concourse.bass as bass
from concourse import bacc
import concourse.mybir as mybir
import concourse.tile as tile
from concourse.bass_utils import run_bass_kernel_spmd

F32 = mybir.dt.float32
F32R = mybir.dt.float32r
MMDT = F32R  # dtype for matmul-feeding activation/weight tiles
AF = mybir.ActivationFunctionType
OP = mybir.AluOpType
VP = mybir.VecI64Pair

L = 384          # sequence length (enc & dec)
NB = 2           # batches per core
T = NB * L       # tokens per core
D = 512
DC = 4           # D / 128
DFF = 2048
FC = 16          # DFF / 128
CIN = 21
MARK = 4
EMBK = 3 * CIN + MARK  # 67
TOPK = 17
CORES = 8
LBL = 192        # label / pred length

_CACHE = {}


def _flat_ap(t, offset, dims):
    """AP into DRAM tensor t at element offset with [step,count] dims."""
    a = t[tuple([slice(0, 1)] * len(t.shape))].copy()
    a.offset = offset
    a.ap = VP(dims)
    return a


class Net:
    def __init__(self, nc, tc, sb, ps, dr):
        self.nc, self.tc, self.sb, self.ps, self.dr = nc, tc, sb, ps, dr
        self.uid = 0

    def nid(self):
        self.uid += 1
        return self.uid

    def big8(self, cols=T):
        return self.sb.tile([128, cols], MMDT, tag="big8", bufs=8,
                            padded_shape=[128, T], name=f"b8_{self.nid()}")

    # -------------------- weights --------------------
    def load_wslice(self, param, col0, ncols, tag, bufs=4):
        t = self.sb.tile([param.shape[0], ncols], param.dtype, tag=tag, bufs=bufs,
                         name=f"w{tag}_{self.nid()}")
        self.nc.gpsimd.dma_start(t[:], param[:, col0:col0 + ncols])
        return t

    # -------------------- series decomp --------------------
    def decomp2d(self, x, p, sea_out):
        """x: SBUF AP [p, 384] one batch. Writes seasonal into sea_out AP."""
        nc, sb = self.nc, self.sb
        cs = sb.tile([p, L], F32, tag="dscr0", bufs=2, name=f"cs_{self.nid()}")
        nc.vector.tensor_tensor_scan(cs[:], x, x, 0.0, op0=OP.add, op1=OP.bypass)
        W = sb.tile([p, L], F32, tag="dscr1", bufs=2, name=f"dw_{self.nid()}")
        # central t in [13, 370]: W[t] = cs[t+12] - cs[t-13]
        nc.vector.tensor_tensor(W[:, 13:371], cs[:, 25:383], cs[:, 0:358],
                                op=OP.subtract)
        # left t in [0, 12]: W[t] = cs[t+12] + (12-t)*x0
        nc.vector.scalar_tensor_tensor(
            W[:, 0:13], self.coefL[0:p, :], x[:, 0:1], cs[:, 12:25],
            op0=OP.mult, op1=OP.add)
        # right t in [371, 383]: W[t] = (t-371)*x383 - (cs[t-13] - cs[383])
        tmp = sb.tile([p, 13], F32, tag="d13", bufs=4, name=f"d13_{self.nid()}")
        nc.vector.tensor_scalar(tmp[:], cs[:, 358:371], cs[:, 383:384], None,
                                op0=OP.subtract)
        nc.vector.scalar_tensor_tensor(
            W[:, 371:384], self.coefR[0:p, :], x[:, 383:384], tmp[:],
            op0=OP.mult, op1=OP.subtract)
        # seasonal = x - W/25
        nc.vector.scalar_tensor_tensor(sea_out, W[:], -1.0 / 25.0, x,
                                       op0=OP.mult, op1=OP.add)

    def decomp_state(self, xt, tsum=None, first=False):
        """xt: 4 tiles [128, 768]. Returns new 4 tiles; if tsum (dict m->tile
        [128,768]) given, trend = x - seasonal is accumulated into it."""
        nc = self.nc
        outs = []
        for m in range(DC):
            newt = self.sb.tile([128, T], MMDT, tag=f"xt{m}", bufs=2,
                                name=f"xs_{self.nid()}")
            for b in range(NB):
                xb = xt[m][:, b * L:(b + 1) * L]
                sb_out = newt[:, b * L:(b + 1) * L]
                self.decomp2d(xb, 128, sb_out)
                if tsum is not None:
                    if first:
                        nc.vector.tensor_tensor(tsum[m][:, b * L:(b + 1) * L],
                                                xb, sb_out, op=OP.subtract)
                    else:
                        tr = self.sb.tile([128, L], MMDT, tag="dtr", bufs=2,
                                          name=f"tr_{self.nid()}")
                        nc.vector.tensor_tensor(tr[:], xb, sb_out,
                                                op=OP.subtract)
                        nc.vector.tensor_add(tsum[m][:, b * L:(b + 1) * L],
                                             tsum[m][:, b * L:(b + 1) * L],
                                             tr[:])
            outs.append(newt)
        return outs

    # -------------------- attention --------------------
    def attention(self, xq, xkv, wq_p, wk_p, wv_p, wo_p, scope):
        """xq/xkv: 4 SBUF tiles [128,768] (transposed acts). Returns 4 new
        tiles = xq + attnout (residual included)."""
        nc, sb, ps, dr = self.nc, self.sb, self.ps, self.dr

        # --- Q^T, K^T projections ([dout_chunk, tokens]) ---
        def proj_T(w_param, src):
            outs = []
            for m in range(DC):
                wt = self.load_wslice(w_param, m * D, D, "wpj")
                ot = self.big8()
                for b in range(NB):
                    pt = ps.tile([128, L], F32, tag="pbig", bufs=4,
                                 name=f"ppj_{self.nid()}")
                    for kc in range(DC):
                        nc.tensor.matmul(
                            pt[:], wt[:, kc * 128:(kc + 1) * 128],
                            src[kc][:, b * L:(b + 1) * L],
                            start=(kc == 0), stop=(kc == DC - 1))
                    nc.scalar.copy(ot[:, b * L:(b + 1) * L], pt[:])
                outs.append(ot)
            return outs

        qt = proj_T(wq_p, xq)
        kt = proj_T(wk_p, xkv)

        # --- V natural [tokens, D] (6 tiles [128, 512]) ---
        vn = {}
        for b in range(NB):
            for tch in range(3):
                pt = ps.tile([128, D], F32, tag="pbig", bufs=4,
                             name=f"pv_{self.nid()}")
                for kc in range(DC):
                    wv_t = self.load_wslice(wv_p, kc * D, D, "wpj")
                    nc.tensor.matmul(
                        pt[:],
                        xkv[kc][:, b * L + tch * 128: b * L + (tch + 1) * 128],
                        wv_t[:],
                        start=(kc == 0), stop=(kc == DC - 1))
                vt = sb.tile([128, D], MMDT, tag="vn", bufs=6,
                             name=f"vn_{self.nid()}")
                nc.scalar.copy(vt[:], pt[:])
                vn[(b, tch)] = vt

        # --- small [1, 384] vectors (engine base-partition rules: keep all
        # at base 0 as separate tiles) ---
        def mvt(nm):
            t = sb.tile([1, L], F32, tag="mv", bufs=11,
                        name=f"{nm}_{self.nid()}")
            return t[:]
        R = {0: mvt("r0"), 1: mvt("r1")}
        mvsum, mvg = mvt("mvsum"), mvt("mvg")
        v2, v3, neg = mvt("v2"), mvt("v3"), mvt("neg")
        mask = sb.tile([1, L], mybir.dt.int32, tag="mvi", bufs=2,
                       name=f"mask_{self.nid()}")[:]
        sel = {b: mvt(f"sel{b}") for b in range(NB)}
        ew = {b: mvt(f"ew{b}") for b in range(NB)}
        wvec = {b: mvt(f"wv{b}") for b in range(NB)}

        # --- Gram diagonal sums -> R[b] = perm(mean_value) [1,384] ---
        for b in range(NB):
            pmv = ps.tile([1, L], F32, tag="psmall", bufs=2,
                          name=f"pmv_{self.nid()}")
            for tch in range(3):
                pg = ps.tile([128, L], F32, tag="pbig", bufs=4,
                             name=f"pg_{self.nid()}")
                for kc in range(DC):
                    nc.tensor.matmul(
                        pg[:],
                        qt[kc][:, b * L + tch * 128: b * L + (tch + 1) * 128],
                        kt[kc][:, b * L:(b + 1) * L],
                        start=(kc == 0), stop=(kc == DC - 1))
                gsb = sb.tile([128, L], F32, tag="gsb", bufs=2,
                              name=f"gsb_{self.nid()}")
                nc.scalar.copy(gsb[:], pg[:])
                gd = dr.tile([128, 2 * L], F32, tag="gdbl", bufs=3,
                             name=f"gd_{self.nid()}")
                nc.gpsimd.dma_start(gd[:, 0:L], gsb[:])
                nc.gpsimd.dma_start(gd[:, L:2 * L], gsb[:])
                sh = sb.tile([128, L], F32, tag="shear", bufs=3,
                             name=f"sh_{self.nid()}")
                nc.gpsimd.dma_start(
                    sh[:], _flat_ap(gd, tch * 128, [[2 * L + 1, 128], [1, L]]))
                nc.tensor.matmul(pmv[:], self.ones_over_d[:], sh[:],
                                 start=(tch == 0), stop=(tch == 2))
            nc.scalar.copy(R[b], pmv[:])

        if getattr(self, "debug", False) and scope == "e0":
            nc.gpsimd.dma_start(self.dbg["dbg_R"][0:1, :], R[0])
            nc.gpsimd.dma_start(self.dbg["dbg_R"][1:2, :], R[1])

        # --- AllReduce of mv sums across 8 cores ---
        nc.vector.tensor_add(mvsum, R[0], R[1])
        cc_in = dr.tile([1, L], F32, tag=f"ccin_{scope}", bufs=1,
                        name=f"ccin_{self.nid()}")
        cc_out = dr.tile([1, L], F32, tag=f"ccout_{scope}", bufs=1,
                         addr_space="Shared", name=f"ccout_{self.nid()}")
        nc.gpsimd.dma_start(cc_in[:], mvsum)
        nc.gpsimd.collective_compute(
            "AllReduce", OP.add, replica_groups=[list(range(CORES))],
            ins=[cc_in[:]], outs=[cc_out[:]])
        nc.gpsimd.dma_start(mvg, cc_out[:])

        # --- tau = 17th largest of mvg (3 rounds of top-8) ---
        m8 = [sb.tile([1, 8], F32, tag="m8", bufs=6,
                      name=f"m8_{self.nid()}") for _ in range(3)]
        nc.vector.max(m8[0][:], mvg)
        nc.vector.match_replace(v2, m8[0][:], mvg, -1e30)
        nc.vector.max(m8[1][:], v2)
        nc.vector.match_replace(v3, m8[1][:], v2, -1e30)
        nc.vector.max(m8[2][:], v3)
        if getattr(self, "debug", False) and scope == "e0":
            nc.gpsimd.dma_start(self.dbg["dbg_mvg"][:], mvg)
            nc.gpsimd.dma_start(self.dbg["dbg_tau"][:], m8[2][:])
        # mask[j] = mvg[j] >= tau  (tau = m8[2][0,0], 17th largest)
        nc.vector.tensor_scalar(mask, mvg, m8[2][0:1, 0:1], None, op0=OP.is_ge)
        nc.vector.memset(neg, -1e30)

        # --- per-batch masked softmax -> w vector; circulant; attn matmul ---
        at = {}
        for b in range(NB):
            nc.vector.select(sel[b], mask, R[b], neg)
            mx, nmx, ssum, rs = [
                sb.tile([1, 1], F32, tag="s1", bufs=8,
                        name=f"s1_{self.nid()}")[:] for _ in range(4)]
            nc.vector.tensor_reduce(mx, sel[b], axis=mybir.AxisListType.X,
                                    op=OP.max)
            nc.vector.tensor_scalar_mul(nmx, mx, -1.0)
            nc.scalar.activation(ew[b], sel[b], AF.Exp, bias=nmx,
                                 scale=1.0, accum_out=ssum)
            nc.vector.reciprocal(rs, ssum)
            nc.vector.tensor_scalar_mul(wvec[b], ew[b], rs)
            # broadcast w to 128 partitions via K=1 matmul, then DRAM replicate
            pbc = ps.tile([128, L], F32, tag="pbig", bufs=4,
                          name=f"pbc_{self.nid()}")
            nc.tensor.matmul(pbc[:], self.ones1[:], wvec[b], start=True,
                             stop=True)
            wbc = sb.tile([128, L], MMDT, tag="wbc", bufs=2,
                          name=f"wbc_{self.nid()}")
            nc.scalar.copy(wbc[:], pbc[:])
            bb = dr.tile([129, L], MMDT, tag="bbig", bufs=2,
                         name=f"bb_{self.nid()}")
            nc.gpsimd.dma_start(bb[0:128, :], wbc[:])
            nc.gpsimd.dma_start(bb[128:129, :], wbc[0:1, :])
            if getattr(self, "debug", False) and scope == "e0":
                nc.gpsimd.dma_start(self.dbg["dbg_w"][b:b + 1, :], wvec[b])
            # S^T tiles: st[tch][p, l] = w[(l - tch*128 - p) mod 384]
            st = []
            for tch in range(3):
                c = (L - tch * 128) % L
                s = sb.tile([128, L], MMDT, tag="shear", bufs=3,
                            name=f"st_{self.nid()}")
                nc.gpsimd.dma_start(s[:], _flat_ap(bb, c, [[L - 1, 128], [1, L]]))
                st.append(s)
            if getattr(self, "debug", False) and scope == "e0" and b == 0:
                nc.gpsimd.dma_start(self.dbg["dbg_st"][:], st[0][:])
            # attn^T[c_chunk, l] = sum_t V[t, c] * S^T[t, l]
            for m in range(DC):
                pa = ps.tile([128, L], F32, tag="pbig", bufs=4,
                             name=f"pa_{self.nid()}")
                for tch in range(3):
                    nc.tensor.matmul(
                        pa[:], vn[(b, tch)][:, m * 128:(m + 1) * 128], st[tch][:],
                        start=(tch == 0), stop=(tch == 2))
                a = self.big8(L)
                nc.scalar.copy(a[:], pa[:])
                at[(b, m)] = a

        # --- Wo projection + residual ---
        newx = []
        for m in range(DC):
            wo_t = self.load_wslice(wo_p, m * D, D, "wpj")
            nt = sb.tile([128, T], MMDT, tag=f"xt{m}", bufs=2,
                         name=f"xa_{self.nid()}")
            for b in range(NB):
                po = ps.tile([128, L], F32, tag="pbig", bufs=4,
                             name=f"po_{self.nid()}")
                for kc in range(DC):
                    nc.tensor.matmul(
                        po[:], wo_t[:, kc * 128:(kc + 1) * 128], at[(b, kc)][:],
                        start=(kc == 0), stop=(kc == DC - 1))
                nc.vector.tensor_add(nt[:, b * L:(b + 1) * L],
                                     xq[m][:, b * L:(b + 1) * L], po[:])
            newx.append(nt)
        return newx

    # -------------------- FFN --------------------
    def ffn(self, xt, w1_p, w2_p):
        nc, sb, ps = self.nc, self.sb, self.ps
        newx = []
        ys = {}
        for b in range(NB):
            yp = [ps.tile([128, L], F32, tag="pbig", bufs=4,
                          name=f"py_{self.nid()}") for _ in range(DC)]
            for kc in range(FC):
                w1t = self.load_wslice(w1_p, kc * D, D, "wffn", bufs=3)
                ph = ps.tile([128, L], F32, tag="pffn", bufs=2,
                             name=f"ph_{self.nid()}")
                for dc in range(DC):
                    nc.tensor.matmul(
                        ph[:], w1t[:, dc * 128:(dc + 1) * 128],
                        xt[dc][:, b * L:(b + 1) * L],
                        start=(dc == 0), stop=(dc == DC - 1))
                h = sb.tile([128, L], MMDT, tag="hff", bufs=3,
                            name=f"h_{self.nid()}")
                nc.scalar.activation(h[:], ph[:], AF.Gelu)
                w2t = self.load_wslice(w2_p, kc * D, D, "wffn", bufs=3)
                for m in range(DC):
                    nc.tensor.matmul(
                        yp[m][:], w2t[:, m * 128:(m + 1) * 128], h[:],
                        start=(kc == 0), stop=(kc == FC - 1))
            ys[b] = yp
        for m in range(DC):
            nt = sb.tile([128, T], MMDT, tag=f"xt{m}", bufs=2,
                         name=f"xf_{self.nid()}")
            for b in range(NB):
                nc.vector.tensor_add(nt[:, b * L:(b + 1) * L],
                                     xt[m][:, b * L:(b + 1) * L], ys[b][m][:])
            newx.append(nt)
        return newx

    # -------------------- my_layernorm --------------------
    def my_layernorm(self, xt, w_param, out_tag, out_bufs):
        nc, sb, ps = self.nc, self.sb, self.ps
        wt = sb.tile([128, DC], F32, tag="wln", bufs=2, name=f"wln_{self.nid()}")
        nc.gpsimd.dma_start(wt[:], w_param[:])
        sq = []
        for m in range(DC):
            s = self.big8()
            nc.scalar.activation(s[:], xt[m][:], AF.Square)
            sq.append(s)
        mean_sb, msq_sb, var, std, rstd = [
            sb.tile([1, T], F32, tag="ln1", bufs=4,
                    name=f"ln_{self.nid()}")[:] for _ in range(5)]
        for half in range(NB):
            pm = ps.tile([1, L], F32, tag="psmall", bufs=2,
                         name=f"plm_{self.nid()}")
            for m in range(DC):
                nc.tensor.matmul(pm[:], self.ood_r[:],
                                 xt[m][:, half * L:(half + 1) * L],
                                 start=(m == 0), stop=(m == DC - 1))
            nc.scalar.copy(mean_sb[0:1, half * L:(half + 1) * L], pm[:])
            pq = ps.tile([1, L], F32, tag="psmall", bufs=2,
                         name=f"plq_{self.nid()}")
            for m in range(DC):
                nc.tensor.matmul(pq[:], self.ood_r[:],
                                 sq[m][:, half * L:(half + 1) * L],
                                 start=(m == 0), stop=(m == DC - 1))
            nc.scalar.copy(msq_sb[0:1, half * L:(half + 1) * L], pq[:])
        nc.vector.tensor_mul(var, mean_sb, mean_sb)
        nc.vector.tensor_tensor(var, msq_sb, var, op=OP.subtract)
        nc.vector.tensor_scalar_add(var, var, 1e-5)
        nc.scalar.activation(std, var, AF.Sqrt)
        nc.vector.reciprocal(rstd, std)
        out = [sb.tile([128, T], MMDT, tag=out_tag.format(m=m), bufs=out_bufs,
                       name=f"xl_{self.nid()}") for m in range(DC)]
        for half in range(NB):
            pmb = ps.tile([128, L], F32, tag="pbig", bufs=4,
                          name=f"pmb_{self.nid()}")
            nc.tensor.matmul(pmb[:], self.ones1[:],
                             mean_sb[0:1, half * L:(half + 1) * L],
                             start=True, stop=True)
            prb = ps.tile([128, L], F32, tag="pbig", bufs=4,
                          name=f"prb_{self.nid()}")
            nc.tensor.matmul(prb[:], self.ones1[:],
                             rstd[0:1, half * L:(half + 1) * L],
                             start=True, stop=True)
            for m in range(DC):
                tmp = sb.tile([128, L], F32, tag="lntmp", bufs=2,
                              name=f"lt_{self.nid()}")
                nc.vector.tensor_tensor(tmp[:], xt[m][:, half * L:(half + 1) * L],
                                        pmb[:], op=OP.subtract)
                nc.vector.scalar_tensor_tensor(
                    out[m][:, half * L:(half + 1) * L], tmp[:], wt[:, m:m + 1],
                    prb[:], op0=OP.mult, op1=OP.mult)
        # subtract per-(d, batch) mean over time
        for m in range(DC):
            red = sb.tile([128, NB], F32, tag="lnred", bufs=3,
                          name=f"lred_{self.nid()}")
            x3 = out[m].rearrange("p (b l) -> p b l", b=NB)
            nc.vector.tensor_reduce(red[:], x3, axis=mybir.AxisListType.X,
                                    op=OP.add)
            nc.vector.tensor_scalar_mul(red[:], red[:], 1.0 / L)
            for b in range(NB):
                nc.vector.tensor_scalar(
                    out[m][:, b * L:(b + 1) * L], out[m][:, b * L:(b + 1) * L],
                    red[:, b:b + 1], None, op0=OP.subtract)
        return out

    # -------------------- embedding --------------------
    def embed(self, xT, xmT, wemb_t, dest, b):
        """xT: [21,384] AP, xmT: [4,384] AP; writes dest[m][:, b*384:+384]."""
        nc, sb, ps = self.nc, self.sb, self.ps
        E = sb.tile([EMBK, L], MMDT, tag="embr", bufs=2, name=f"E_{self.nid()}")
        nc.gpsimd.dma_start(E[0:CIN, 1:L], xT[:, 0:L - 1])
        nc.gpsimd.dma_start(E[0:CIN, 0:1], xT[:, L - 1:L])
        nc.gpsimd.dma_start(E[CIN:2 * CIN, :], xT)
        nc.gpsimd.dma_start(E[2 * CIN:3 * CIN, 0:L - 1], xT[:, 1:L])
        nc.gpsimd.dma_start(E[2 * CIN:3 * CIN, L - 1:L], xT[:, 0:1])
        nc.gpsimd.dma_start(E[3 * CIN:EMBK, :], xmT)
        for m in range(DC):
            pe = ps.tile([128, L], F32, tag="pbig", bufs=4,
                         name=f"pe_{self.nid()}")
            nc.tensor.matmul(pe[:], wemb_t[:, m * 128:(m + 1) * 128], E[:],
                             start=True, stop=True)
            nc.scalar.copy(dest[m][:, b * L:(b + 1) * L], pe[:])


def build_nc(debug=False):
    nc = bacc.Bacc(None, target_bir_lowering=False)

    # ---- I/O ----
    x_enc = nc.declare_dram_parameter("x_enc", [NB, L, CIN], F32, isOutput=False)
    xm_enc = nc.declare_dram_parameter("xm_enc", [NB, L, MARK], F32, isOutput=False)
    xm_dec = nc.declare_dram_parameter("xm_dec", [NB, L, MARK], F32, isOutput=False)
    out = nc.declare_dram_parameter("out", [NB, LBL, CIN], F32, isOutput=True)

    dbg = {}
    if debug:
        for nm, shape in [
            ("dbg_xt", [64, L]), ("dbg_emb", [D, T]), ("dbg_R", [NB, L]),
            ("dbg_mvg", [1, L]), ("dbg_tau", [1, 8]), ("dbg_w", [NB, L]),
            ("dbg_st", [128, L]), ("dbg_attn", [D, T]), ("dbg_dc1", [D, T]),
            ("dbg_ffn", [D, T]), ("dbg_enc", [D, T]), ("dbg_ti", [NB * CIN, L]),
            ("dbg_si", [NB * CIN, L]), ("dbg_decemb", [D, T]),
            ("dbg_dec1", [D, T]), ("dbg_tsum", [D, T]), ("dbg_ta", [NB * CIN, L]),
            ("dbg_decn", [D, T]),
        ]:
            dbg[nm] = nc.declare_dram_parameter(nm, shape, F32, isOutput=True)

    def wparam(name, shape, dt=MMDT):
        return nc.declare_dram_parameter(name, shape, dt, isOutput=False)

    wz = {}
    for li in range(2):
        for wn in ("wq", "wk", "wv", "wo"):
            wz[f"enc{li}_{wn}"] = wparam(f"enc{li}_{wn}", [128, 4 * D])
        wz[f"enc{li}_w1"] = wparam(f"enc{li}_w1", [128, FC * D])
        wz[f"enc{li}_w2"] = wparam(f"enc{li}_w2", [128, FC * D])
    for pre in ("dself", "dcross"):
        for wn in ("wq", "wk", "wv", "wo"):
            wz[f"{pre}_{wn}"] = wparam(f"{pre}_{wn}", [128, 4 * D])
    wz["dec_w1"] = wparam("dec_w1", [128, FC * D])
    wz["dec_w2"] = wparam("dec_w2", [128, FC * D])
    wz["dec_wtrend"] = wparam("dec_wtrend", [128, 3 * DC * CIN])
    wz["enc_emb"] = wparam("enc_emb", [EMBK, D])
    wz["dec_emb"] = wparam("dec_emb", [EMBK, D])
    wz["enc_norm_w"] = wparam("enc_norm_w", [128, DC], F32)
    wz["dec_norm_w"] = wparam("dec_norm_w", [128, DC], F32)
    wz["proj_w"] = wparam("proj_w", [128, DC * CIN])
    wz["consts"] = wparam("consts", [128, 1 + 128 + 128 + 13 + 13], F32)
    wz["ood_r"] = wparam("ood_r", [128, 1])

    with tile.TileContext(nc) as tc:
        with (
            tc.tile_pool(name="sb", bufs=1) as sb,
            tc.tile_pool(name="ps", bufs=1, space="PSUM") as ps,
            tc.tile_pool(name="dr", bufs=1, space="DRAM") as dr,
        ):
            net = Net(nc, tc, sb, ps, dr)

            def dump_state(nm, xt):
                if debug:
                    for m in range(DC):
                        nc.gpsimd.dma_start(dbg[nm][m * 128:(m + 1) * 128, :],
                                            xt[m][:])

            def dump(nm, ap, row0=0, nrows=None):
                if debug:
                    nrows = nrows if nrows is not None else ap.shape[0]
                    nc.gpsimd.dma_start(dbg[nm][row0:row0 + nrows, :], ap)
            net.debug = debug
            net.dbg = dbg

            # ---- consts ----
            cst = sb.tile([128, 1 + 128 + 128 + 13 + 13], F32, tag="cst", bufs=1)
            nc.gpsimd.dma_start(cst[:], wz["consts"][:])
            net.ones_over_d = cst[:, 0:1]             # [128,1] = 1/512
            net.ones1 = cst[0:1, 1:129]               # [1,128] ones
            net.ident = cst[:, 129:257]               # [128,128] identity
            net.coefL = cst[:, 257:270]               # [128,13]
            net.coefR = cst[:, 270:283]               # [128,13]
            oodr = sb.tile([128, 1], MMDT, tag="oodr", bufs=1)
            nc.gpsimd.dma_start(oodr[:], wz["ood_r"][:])
            net.ood_r = oodr

            # ---- input transposes ----
            # per t-chunk: [128, 64] -> T -> [64, 128]; batch b at row-base
            # 32*b so engine ops on x_enc^T rows stay 32-aligned.
            xt_sb = sb.tile([64, L], F32, tag="xts", bufs=1)
            for tch in range(3):
                tin = sb.tile([128, 64], F32, tag="tin", bufs=2,
                              name=f"tin_{tch}")
                for b in range(NB):
                    c0 = b * 32
                    nc.gpsimd.dma_start(
                        tin[:, c0:c0 + CIN],
                        x_enc[b, tch * 128:(tch + 1) * 128, :])
                    nc.gpsimd.dma_start(
                        tin[:, c0 + CIN:c0 + CIN + MARK],
                        xm_enc[b, tch * 128:(tch + 1) * 128, :])
                    nc.gpsimd.dma_start(
                        tin[:, c0 + 25:c0 + 29],
                        xm_dec[b, tch * 128:(tch + 1) * 128, :])
                ptp = ps.tile([64, 128], F32, tag="psmall", bufs=2,
                              name=f"ptp_{tch}")
                nc.tensor.transpose(ptp[:], tin[:], net.ident)
                nc.scalar.copy(xt_sb[:, tch * 128:(tch + 1) * 128], ptp[:])

            dump("dbg_xt", xt_sb[:])

            # ---- initial decomp on x_enc^T; trend/seasonal inits ----
            ti, si = [], []
            xe = []
            for b in range(NB):
                xeb = sb.tile([CIN, L], F32, tag=f"xe{b}", bufs=1,
                              name=f"xe_{b}")
                nc.vector.tensor_copy(xeb[:], xt_sb[b * 32: b * 32 + CIN, :])
                xe.append(xeb)
            for b in range(NB):
                rows = xe[b][:]
                sea = sb.tile([CIN, L], F32, tag="init21", bufs=2,
                              name=f"isea_{b}")
                net.decomp2d(rows, CIN, sea[:])
                tr = sb.tile([CIN, L], F32, tag="init21", bufs=2,
                             name=f"itr_{b}")
                nc.vector.tensor_tensor(tr[:], rows, sea[:], op=OP.subtract)
                mean = sb.tile([CIN, 1], F32, tag="im", bufs=2,
                               name=f"imean_{b}")
                nc.vector.tensor_reduce(mean[:], rows, axis=mybir.AxisListType.X,
                                        op=OP.add)
                nc.vector.tensor_scalar_mul(mean[:], mean[:], 1.0 / L)
                tib = sb.tile([CIN, L], F32, tag=f"ti{b}", bufs=1,
                              name=f"ti_{b}")
                nc.vector.tensor_copy(tib[:, 0:LBL], tr[:, LBL:L])
                nc.scalar.activation(tib[:, LBL:L], tr[:, 0:LBL], AF.Identity,
                                     scale=0.0, bias=mean[:, 0:1])
                ti.append(tib)
                sib = sb.tile([CIN, L], F32, tag=f"si{b}", bufs=1,
                              name=f"si_{b}")
                nc.vector.tensor_copy(sib[:, 0:LBL], sea[:, LBL:L])
                nc.vector.memset(sib[:, LBL:L], 0.0)
                si.append(sib)

            if debug:
                for b in range(NB):
                    dump("dbg_ti", ti[b][:], row0=b * CIN, nrows=CIN)
                    dump("dbg_si", si[b][:], row0=b * CIN, nrows=CIN)

            # ---- encoder ----
            wemb_e = sb.tile([EMBK, D], MMDT, tag="wemb", bufs=2, name="wembe")
            nc.gpsimd.dma_start(wemb_e[:], wz["enc_emb"][:])
            enc = [sb.tile([128, T], MMDT, tag=f"xt{m}", bufs=2,
                           name=f"enc0_{m}") for m in range(DC)]
            for b in range(NB):
                net.embed(xe[b][:], xt_sb[b * 32 + CIN:b * 32 + CIN + MARK, :],
                          wemb_e, enc, b)

            dump_state("dbg_emb", enc)
            for li in range(2):
                with nc.named_scope(f"enc{li}_attn"):
                    enc = net.attention(enc, enc, wz[f"enc{li}_wq"],
                                        wz[f"enc{li}_wk"], wz[f"enc{li}_wv"],
                                        wz[f"enc{li}_wo"], f"e{li}")
                if debug and li == 0:
                    dump_state("dbg_attn", enc)
                with nc.named_scope(f"enc{li}_decomp1"):
                    enc = net.decomp_state(enc)
                if debug and li == 0:
                    dump_state("dbg_dc1", enc)
                with nc.named_scope(f"enc{li}_ffn"):
                    enc = net.ffn(enc, wz[f"enc{li}_w1"], wz[f"enc{li}_w2"])
                if debug and li == 0:
                    dump_state("dbg_ffn", enc)
                with nc.named_scope(f"enc{li}_decomp2"):
                    enc = net.decomp_state(enc)
            with nc.named_scope("enc_norm"):
                enc = net.my_layernorm(enc, wz["enc_norm_w"], "enc{m}", 1)

            dump_state("dbg_enc", enc)

            # ---- decoder ----
            wemb_d = sb.tile([EMBK, D], MMDT, tag="wemb", bufs=2, name="wembd")
            nc.gpsimd.dma_start(wemb_d[:], wz["dec_emb"][:])
            dec = [sb.tile([128, T], MMDT, tag=f"xt{m}", bufs=2,
                           name=f"dec0_{m}") for m in range(DC)]
            for b in range(NB):
                net.embed(si[b][:], xt_sb[b * 32 + 25:b * 32 + 29, :],
                          wemb_d, dec, b)

            dump_state("dbg_decemb", dec)
            tsum = {m: sb.tile([128, T], MMDT, tag=f"ts{m}", bufs=1,
                               name=f"ts_{m}") for m in range(DC)}
            with nc.named_scope("dec_self"):
                dec = net.attention(dec, dec, wz["dself_wq"], wz["dself_wk"],
                                    wz["dself_wv"], wz["dself_wo"], "ds")
            if debug:
                dump_state("dbg_dec1", dec)
            dec = net.decomp_state(dec, tsum, first=True)
            with nc.named_scope("dec_cross"):
                dec = net.attention(dec, enc, wz["dcross_wq"], wz["dcross_wk"],
                                    wz["dcross_wv"], wz["dcross_wo"], "dc")
            dec = net.decomp_state(dec, tsum)
            with nc.named_scope("dec_ffn"):
                dec = net.ffn(dec, wz["dec_w1"], wz["dec_w2"])
            dec = net.decomp_state(dec, tsum)

            if debug:
                dump_state("dbg_tsum", [tsum[m] for m in range(DC)])

            # ---- trend conv: rt = circ_conv3(t1+t2+t3, Wtrend) ----
            wtr = sb.tile([128, 3 * DC * CIN], MMDT, tag="wtr", bufs=1,
                          name="wtrend")
            nc.gpsimd.dma_start(wtr[:], wz["dec_wtrend"][:])
            ta = []
            for b in range(NB):
                td = []
                for m in range(DC):
                    t2d = sb.tile([128, L + 2], MMDT, tag="tdb", bufs=4,
                                  name=f"td_{net.nid()}")
                    src = tsum[m][:, b * L:(b + 1) * L]
                    nc.vector.tensor_copy(t2d[:, 1:L + 1], src)
                    nc.vector.tensor_copy(t2d[:, 0:1],
                                          tsum[m][:, b * L + L - 1:b * L + L])
                    nc.vector.tensor_copy(t2d[:, L + 1:L + 2],
                                          tsum[m][:, b * L:b * L + 1])
                    td.append(t2d)
                prt = ps.tile([CIN, L], F32, tag="psmall", bufs=2,
                              name=f"prt_{b}")
                k = 0
                for j in range(3):
                    for m in range(DC):
                        nc.tensor.matmul(
                            prt[:], wtr[:, (j * DC + m) * CIN:(j * DC + m + 1) * CIN],
                            td[m][:, j:j + L],
                            start=(k == 0), stop=(k == 3 * DC - 1))
                        k += 1
                tab = sb.tile([CIN, L], F32, tag=f"ta{b}", bufs=1,
                              name=f"ta_{b}")
                nc.vector.tensor_add(tab[:], ti[b][:], prt[:])
                ta.append(tab)

            if debug:
                for b in range(NB):
                    dump("dbg_ta", ta[b][:], row0=b * CIN, nrows=CIN)

            # ---- final norm + projection + output ----
            with nc.named_scope("dec_norm"):
                dec = net.my_layernorm(dec, wz["dec_norm_w"], "xt{m}", 2)
            dump_state("dbg_decn", dec)
            pw = sb.tile([128, DC * CIN], MMDT, tag="wpr", bufs=1, name="projw")
            nc.gpsimd.dma_start(pw[:], wz["proj_w"][:])
            for b in range(NB):
                psp = ps.tile([CIN, L], F32, tag="psmall", bufs=2,
                              name=f"psp_{b}")
                for m in range(DC):
                    nc.tensor.matmul(psp[:], pw[:, m * CIN:(m + 1) * CIN],
                                     dec[m][:, b * L:(b + 1) * L],
                                     start=(m == 0), stop=(m == DC - 1))
                tot = sb.tile([CIN, L], F32, tag="tot", bufs=2,
                              name=f"tot_{b}")
                nc.vector.tensor_add(tot[:], ta[b][:], psp[:])
                for h in range(2):
                    ptr = ps.tile([96, CIN], F32, tag="psmall", bufs=2,
                                  name=f"ptr_{b}_{h}")
                    nc.tensor.transpose(ptr[:], tot[:, LBL + h * 96:LBL + (h + 1) * 96],
                                        net.ident[0:CIN, 0:CIN])
                    osb = sb.tile([96, CIN], F32, tag="osb", bufs=2,
                                  name=f"osb_{b}_{h}")
                    nc.scalar.copy(osb[:], ptr[:])
                    nc.gpsimd.dma_start(out[b, h * 96:(h + 1) * 96, :], osb[:])

    nc.compile()
    return nc


# ----------------------------------------------------------------------
# host side
# ----------------------------------------------------------------------

def _prep_weights(params):
    g = lambda a: np.asarray(a, dtype=np.float32)

    def mmajor(W):  # [512,512] -> [128, 4*512], m-major (for ^T projections)
        return np.ascontiguousarray(
            g(W).reshape(4, 128, 4, 128).transpose(1, 2, 0, 3).reshape(128, 2048))

    def cmajor(W):  # [512,512] -> [128, 4*512], chunk-major (for V natural rhs)
        return np.ascontiguousarray(
            g(W).reshape(4, 128, 512).transpose(1, 0, 2).reshape(128, 2048))

    def w1fmt(W):  # [512,2048] -> [128, 16*512] kc-major with dc minor
        return np.ascontiguousarray(
            g(W).reshape(4, 128, 16, 128).transpose(1, 2, 0, 3).reshape(128, 8192))

    def w2fmt(W):  # [2048,512] -> [128, 16*512] kc-major
        return np.ascontiguousarray(
            g(W).reshape(16, 128, 512).transpose(1, 0, 2).reshape(128, 8192))

    def lnfmt(w):  # [512] -> [128, 4]
        return np.ascontiguousarray(g(w).reshape(4, 128).T)

    out = {}
    for li in range(2):
        p = params["enc_layers"][li]
        out[f"enc{li}_wq"] = mmajor(p["attn"]["Wq"])
        out[f"enc{li}_wk"] = mmajor(p["attn"]["Wk"])
        out[f"enc{li}_wv"] = cmajor(p["attn"]["Wv"])
        out[f"enc{li}_wo"] = mmajor(p["attn"]["Wo"])
        out[f"enc{li}_w1"] = w1fmt(p["W1"])
        out[f"enc{li}_w2"] = w2fmt(p["W2"])
    dp = params["dec_layers"][0]
    for pre, key in (("dself", "self"), ("dcross", "cross")):
        out[f"{pre}_wq"] = mmajor(dp[key]["Wq"])
        out[f"{pre}_wk"] = mmajor(dp[key]["Wk"])
        out[f"{pre}_wv"] = cmajor(dp[key]["Wv"])
        out[f"{pre}_wo"] = mmajor(dp[key]["Wo"])
    out["dec_w1"] = w1fmt(dp["W1"])
    out["dec_w2"] = w2fmt(dp["W2"])
    # Wtrend [3, 512, 21] -> [128, 3*4*21], (j, dc)-major
    out["dec_wtrend"] = np.ascontiguousarray(
        g(dp["Wtrend"]).reshape(3, 4, 128, CIN).transpose(2, 0, 1, 3)
        .reshape(128, 3 * DC * CIN))
    for side in ("enc", "dec"):
        e = params[f"{side}_emb"]
        wt = g(e["Wtok"])  # [3, 21, 512]
        out[f"{side}_emb"] = np.ascontiguousarray(
            np.concatenate([wt[0], wt[1], wt[2], g(e["Wtemp"])], axis=0))
    out["enc_norm_w"] = lnfmt(params["enc_norm_w"])
    out["dec_norm_w"] = lnfmt(params["dec_norm_w"])
    out["proj_w"] = np.ascontiguousarray(
        g(params["proj_W"]).reshape(4, 128, CIN).transpose(1, 0, 2)
        .reshape(128, DC * CIN))
    # consts: [1/512 col | ones row block | identity | coefL | coefR]
    cst = np.zeros((128, 1 + 128 + 128 + 13 + 13), np.float32)
    cst[:, 0] = 1.0 / D
    cst[0, 1:129] = 1.0
    cst[:, 129:257] = np.eye(128, dtype=np.float32)
    cst[:, 257:270] = np.arange(12, -1, -1, dtype=np.float32)[None, :]
    cst[:, 270:283] = np.arange(0, 13, dtype=np.float32)[None, :]
    out["consts"] = cst
    out["ood_r"] = np.full((128, 1), 1.0 / D, np.float32)
    return out


def kernel(x_enc, x_mark_enc, x_dec, x_mark_dec, params):
    if "nc" not in _CACHE:
        _CACHE["nc"] = build_nc()
    nc = _CACHE["nc"]

    wmaps = _prep_weights(params)
    x_enc = np.ascontiguousarray(np.asarray(x_enc, dtype=np.float32))
    xme = np.ascontiguousarray(np.asarray(x_mark_enc, dtype=np.float32))
    xmd = np.ascontiguousarray(np.asarray(x_mark_dec, dtype=np.float32))

    in_maps = []
    for c in range(CORES):
        m = dict(wmaps)
        m["x_enc"] = x_enc[2 * c:2 * c + 2]
        m["xm_enc"] = xme[2 * c:2 * c + 2]
        m["xm_dec"] = xmd[2 * c:2 * c + 2]
        in_maps.append(m)

    trace = bool(_CACHE.get("trace"))
    res = run_bass_kernel_spmd(nc, in_maps, core_ids=list(range(CORES)),
                               trace=trace,
                               stitch_traces=trace,
                               trace_cores=list(range(CORES)) if trace else None)
    if trace:
        _CACHE["last_result"] = res
    return np.concatenate([res.results[c]["out"] for c in range(CORES)], axis=0)
